# revision 20
# baseline (speedup 1.0000x reference)
"""Trainium2 Bass kernel for a 3-layer FCL + size-5 sliding-window stack.

Reference computation (fp32):
    h = relu(x @ W1.T)          # [N, 10]
    t = relu(h @ W2.T + b2)     # [N, 5]
    out[n] = concat(t[n-2..n+2])  zero-padded  -> [N, 25]

Strategy (8 cores, data-parallel over rows, halo recomputed per core):
  - Each core gets a 25088-row shard (25000 own rows + 2-row halo each side,
    zero padded, rounded up to 49 blocks of 512 rows).
  - Per 512-row block: x is DMA-loaded with a fused f32->bf16 cast (SWDGE),
    rows (4p+j) on partition p so the HBM read is 5120 contiguous bytes per
    partition.  bf16 tiles are transposed on the tensor engine so the
    320-dim contraction sits on partitions (bf16 streams 1 col/cycle vs 2
    for fp32); L1 accumulates hT[10,512] in fp32 PSUM; L2 multiplies back
    to natural layout t[128, 4x5]; bias on DVE, relu on ACT.
  - t rows stream to a DRAM scratch t_buf[25088, 5] (fp32).  Output row n is
    the contiguous 25-element window t_buf.flat[5n:5n+25]: each partition
    DMA-loads one contiguous 8-row strip (160B descriptors), DVE expands the
    five overlapping windows via a strided read AP, and the result stores
    with 400B-per-partition descriptors.  Gathers lag two blocks so their
    waits are pre-satisfied and never stall the queues.
  - The ISA allows ONE sync-wait per instruction and Tile does not split
    multi-waits: engines are choreographed so nearly every instruction has
    at most one unobserved producer, and a post-pass hoists any remaining
    extra waits onto same-engine NoOps.
  - Global zero-padding of the window (4 tiny row slices at the ends of the
    full output) is patched on the host during unsharding.
"""

import numpy as np

import bass_rust
import concourse.bass as bass
import concourse.mybir as mybir
import concourse.tile as tile
from concourse.tile import add_dep_helper

# ---- problem constants (hardcoded per contract) ----
N = 200000
D = 320
D1 = 10
D2 = 5
W = 5
HALF = W // 2
NCORES = 8
ROWS = N // NCORES          # 25000 output rows per core
BLK = 512                   # rows per compute block
JJ = 4                      # rows per partition per block (512 = 128*4)
NBLK = 49                   # ceil((ROWS + 4) / BLK) -> 25088 padded rows
PAD = NBLK * BLK            # 25088
CHUNKS = [(0, 128), (128, 128), (256, 64)]  # d-chunks of 320
F32 = mybir.dt.float32
BF16 = mybir.dt.bfloat16
RELU = mybir.ActivationFunctionType.Relu

_NC_CACHE = {}


def _dep(a, b, why):
    add_dep_helper(a.ins, b.ins, reason=why)


def split_multiwaits(nc):
    """Walrus/ISA allows ONE sync-wait per instruction; Tile emits several.

    For every instruction with >1 wait, hoist all but the last wait onto
    fresh NoOps on the same engine immediately before it.  The engine
    stalls at the nops exactly as it would have at the instruction, so
    semantics are unchanged.
    """
    n_split = 0
    for bb in nc.main_func.blocks:
        insts = bb.instructions
        out = []
        changed = False
        for ins in insts:
            si = ins.sync_info
            waits = list(si.on_wait) if si is not None else []
            if len(waits) > 1:
                changed = True
                for w in waits[:-1]:
                    n_split += 1
                    nop = bass_rust.InstNoOp(name=f"wsplit-{n_split}")
                    nop.engine = ins.engine
                    nop.sync_info = bass_rust.SyncInfo(
                        on_wait=[w], on_update=[]
                    )
                    nc.inst_map[nop.name] = nop
                    out.append(nop)
                ins.sync_info = bass_rust.SyncInfo(
                    on_wait=[waits[-1]], on_update=list(si.on_update)
                )
            out.append(ins)
        if changed:
            bb.instructions = out
    return n_split


def build_nc():
    nc = bass.Bass("TRN2", target_bir_lowering=False, debug=False)

    x_t = nc.dram_tensor("x", [PAD, D], F32, kind="ExternalInput")
    w1_t = nc.dram_tensor("W1", [D1, D], F32, kind="ExternalInput")
    w2_t = nc.dram_tensor("W2", [D2, D1], F32, kind="ExternalInput")
    b2_t = nc.dram_tensor("b2", [D2], F32, kind="ExternalInput")
    out_t = nc.dram_tensor("out", [ROWS, W * D2], F32, kind="ExternalOutput")
    tbuf_t = nc.dram_tensor("t_buf", [PAD, D2], F32)  # internal scratch

    with tile.TileContext(nc) as tc:
        with (
            tc.tile_pool(name="singles", bufs=1) as singles,
            tc.tile_pool(name="xpool", bufs=6) as xpool,
            tc.tile_pool(name="xtpool", bufs=8) as xtpool,
            tc.tile_pool(name="hpool", bufs=5) as hpool,
            tc.tile_pool(name="tpool", bufs=NBLK) as tpool,
            tc.tile_pool(name="wpool", bufs=NBLK) as wpool,
            tc.tile_pool(name="gpool", bufs=NBLK) as gpool,
            tc.tile_pool(name="ps_xt", bufs=4, space="PSUM") as ps_xt,
            tc.tile_pool(name="ps_h", bufs=2, space="PSUM") as ps_h,
            tc.tile_pool(name="ps_t", bufs=2, space="PSUM") as ps_t,
        ):
            # ---- constants ----
            ident = singles.tile([128, 128], BF16)
            nc.gpsimd.memset(ident, 0.0)
            asel = nc.gpsimd.affine_select(
                out=ident,
                in_=ident,
                compare_op=mybir.AluOpType.not_equal,
                fill=1.0,
                base=0,
                pattern=[[-1, 128]],
                channel_multiplier=1,
            )
            w1_sb = singles.tile([D1, D], F32)
            nc.sync.dma_start(out=w1_sb, in_=w1_t[:, :])
            w2_sb = singles.tile([D2, D1], F32)
            nc.sync.dma_start(out=w2_sb, in_=w2_t[:, :])
            # b2 replicated across partitions and the 4 row-subtiles
            b2rep = singles.tile([128, JJ, D2], F32)
            b2dma = nc.gpsimd.dma_start(
                out=b2rep, in_=bass.AP(b2_t, 0, [[0, 128], [0, JJ], [1, D2]])
            )

            # bf16 casts of the weights (DVE), then PE transposes.
            w1_bf = singles.tile([D1, D], BF16)
            nc.vector.tensor_copy(out=w1_bf, in_=w1_sb)
            w2_bf = singles.tile([D2, D1], BF16)
            nc.vector.tensor_copy(out=w2_bf, in_=w2_sb)

            # PE observes the identity build once; transposes then only wait
            # on their data producer.
            nop_id = nc.tensor.nop()
            _dep(nop_id, asel, "PE pre-observe identity")

            w2t_sb = singles.tile([D1, D2], BF16)
            wps = ps_xt.tile([128, BLK], F32, tag="xt", name="wps_w2")
            nc.tensor.matmul(wps[:D1, :D2], w2_bf[:, :], ident[:D2, :D2], start=True, stop=True)
            nc.scalar.copy(out=w2t_sb, in_=wps[:D1, :D2])

            w1t_sb = singles.tile([128, len(CHUNKS), D1], BF16)
            last_wcopy = None
            for c, (d0, cw) in enumerate(CHUNKS):
                wps = ps_xt.tile([128, BLK], F32, tag="xt", name=f"wps_{c}")
                nc.tensor.matmul(
                    wps[:cw, :D1],
                    w1_bf[:, d0 : d0 + cw],
                    ident[:D1, :D1],
                    start=True,
                    stop=True,
                )
                last_wcopy = nc.scalar.copy(
                    out=w1t_sb[:cw, c, :], in_=wps[:cw, :D1]
                )
            # PE observes the weight copies (ACT) once.
            nop_w = nc.tensor.nop()
            _dep(nop_w, last_wcopy, "PE pre-observe W1T/W2T copies")
            # DVE observes the b2 broadcast once.
            nop_b2 = nc.vector.nop()
            _dep(nop_b2, b2dma, "DVE pre-observe b2 broadcast")

            stores = {}
            gwins = {}
            gready = {}

            def emit_gather_load(g):
                """t_buf -> SBUF strips + DVE window expansion.

                Partition p loads the contiguous 8 rows [512g+4p, 512g+4p+8)
                (one 160B descriptor per partition); window w of output row
                512g+4p+j is strip elements [5(j+w), 5(j+w)+5) - an affine
                overlapping read the DVE expands into [128, JJ, 25].
                RAW deps are stores g and g+1 (two DMAHW lanes): a sync nop
                observes store g so the DMA itself waits only on store g+1.
                """
                nrows = min(BLK, ROWS - BLK * g)
                npart = nrows // JJ
                nop_g = nc.sync.nop()
                _dep(nop_g, stores[g], "SP pre-observe t store g")
                win_sb = wpool.tile([128, (JJ + W - 1) * D2], F32, tag="w")
                nc.sync.dma_start(
                    out=win_sb[:npart],
                    in_=bass.AP(
                        tbuf_t,
                        BLK * g * D2,
                        [[JJ * D2, npart], [1, (JJ + W - 1) * D2]],
                    ),
                )
                g_sb = gpool.tile([128, JJ, W * D2], F32, tag="g")
                last = None
                for j in range(JJ):
                    last = nc.vector.tensor_copy(
                        out=g_sb[:npart, j, :],
                        in_=bass.AP(
                            win_sb.tensor,
                            win_sb.offset + j * D2,
                            [[win_sb.ap[0][0], npart], [D2, W], [1, D2]],
                        ),
                    )
                gwins[g] = (g_sb, npart)
                gready[g] = last

            def emit_gather_store(g):
                g_sb, npart = gwins[g]
                nc.sync.dma_start(
                    out=bass.AP(
                        out_t,
                        BLK * g * W * D2,
                        [[JJ * W * D2, npart], [W * D2, JJ], [1, W * D2]],
                    ),
                    in_=g_sb[:npart],
                )

            def emit_tail(b, h_sbs):
                """L2 + bias/relu + t store for block b (lagged one block
                so the L2 matmuls never stall the PE queue on a fresh relu)."""
                h_sb = h_sbs[b]
                t_ps = ps_t.tile([128, JJ, D2], F32, tag="t")
                for j in range(JJ):
                    nc.tensor.matmul(
                        t_ps[:, j, :],
                        h_sb[:, j * 128 : (j + 1) * 128],
                        w2t_sb,
                        start=True,
                        stop=True,
                    )
                t_sb = tpool.tile([128, JJ, D2], F32, tag="ts")
                nc.vector.tensor_add(t_sb, t_ps, b2rep)
                nc.scalar.activation(t_sb, t_sb, RELU)
                stores[b] = nc.sync.dma_start(
                    out=bass.AP(
                        tbuf_t, b * BLK * D2, [[JJ * D2, 128], [D2, JJ], [1, D2]]
                    ),
                    in_=t_sb,
                )

            # ---- main loop over 512-row blocks (software-pipelined) ----
            h_sbs = {}
            for b in range(NBLK):
                # rows [512b, 512b+512): partition p holds rows 4p+j as
                # contiguous 5120B reads, cast f32->bf16 in the DMA (SWDGE).
                x_sb = xpool.tile([128, JJ, D], BF16, tag="x")
                for j in range(JJ):
                    nc.gpsimd.dma_start(
                        out=x_sb[:, j, :],
                        in_=bass.AP(
                            x_t,
                            b * BLK * D + j * D,
                            [[JJ * D, 128], [1, D]],
                        ),
                    )

                # all 12 transposes first: the PE FIFO never blocks on a
                # PSUM->SBUF copy while transposes are still runnable.
                xt_pss = []
                xt_sbs = []
                for c, (d0, cw) in enumerate(CHUNKS):
                    xt_ps = ps_xt.tile([128, BLK], BF16, tag="xt")
                    xt_sb = xtpool.tile([128, BLK], BF16, tag="xts")
                    for j in range(JJ):
                        nc.tensor.transpose(
                            xt_ps[:cw, j * 128 : (j + 1) * 128],
                            x_sb[:, j, d0 : d0 + cw],
                            ident,
                        )
                    if c == 1:
                        nc.vector.tensor_copy(out=xt_sb[:cw], in_=xt_ps[:cw])
                    else:
                        nc.scalar.copy(out=xt_sb[:cw], in_=xt_ps[:cw])
                    xt_pss.append(xt_ps)
                    xt_sbs.append(xt_sb)

                h_ps = ps_h.tile([D1, BLK], F32, tag="h")
                for c, (d0, cw) in enumerate(CHUNKS):
                    nc.tensor.matmul(
                        h_ps,
                        w1t_sb[:cw, c, :],
                        xt_sbs[c][:cw],
                        start=(c == 0),
                        stop=(c == len(CHUNKS) - 1),
                    )

                h_sb = hpool.tile([D1, BLK], BF16, tag="hs")
                nc.scalar.activation(h_sb, h_ps, RELU)
                h_sbs[b] = h_sb

                if b >= 1:
                    emit_tail(b - 1, h_sbs)
                if b >= 3:
                    emit_gather_load(b - 3)
                if b >= 4:
                    emit_gather_store(b - 4)

            emit_tail(NBLK - 1, h_sbs)
            for g in (NBLK - 3, NBLK - 2, NBLK - 1):
                emit_gather_load(g)
            for g in (NBLK - 4, NBLK - 3, NBLK - 2, NBLK - 1):
                emit_gather_store(g)

    split_multiwaits(nc)
    return nc


def make_shards(x):
    """Per-core [PAD, D] shards with +-2 halo rows, zero padded."""
    shards = []
    for c in range(NCORES):
        s = np.zeros((PAD, D), dtype=np.float32)
        lo = ROWS * c - HALF
        hi = ROWS * c + ROWS + HALF
        src_lo, src_hi = max(lo, 0), min(hi, N)
        s[src_lo - lo : src_lo - lo + (src_hi - src_lo)] = x[src_lo:src_hi]
        shards.append(s)
    return shards


def _patch_edges(out):
    # the reference zero-pads t, not x: window slots that fall outside
    # [0, N) must be exactly zero.
    out[0, : 2 * D2] = 0.0
    out[1, :D2] = 0.0
    out[N - 2, 4 * D2 :] = 0.0
    out[N - 1, 3 * D2 :] = 0.0
    return out


def run(inputs, trace=False):
    from concourse.bass_utils import run_bass_kernel_spmd

    x = np.ascontiguousarray(np.asarray(inputs["x"], dtype=np.float32))
    W1 = np.ascontiguousarray(np.asarray(inputs["W1"], dtype=np.float32))
    W2 = np.ascontiguousarray(np.asarray(inputs["W2"], dtype=np.float32))
    b2 = np.ascontiguousarray(np.asarray(inputs["b2"], dtype=np.float32))
    assert x.shape == (N, D)

    if "nc" not in _NC_CACHE:
        _NC_CACHE["nc"] = build_nc()
    nc = _NC_CACHE["nc"]

    in_maps = [{"x": s, "W1": W1, "W2": W2, "b2": b2} for s in make_shards(x)]
    res = run_bass_kernel_spmd(nc, in_maps, list(range(NCORES)), trace=trace)
    out = np.concatenate([res.results[c]["out"] for c in range(NCORES)], axis=0)
    return _patch_edges(out), res


def kernel(**inputs):
    out, _ = run(inputs, trace=False)
    return out
